# revision 4
# baseline (speedup 1.0000x reference)
"""ColBERT negative-CE loss on 8 Trainium2 NeuronCores (Bass/Tile).

Problem (hardcoded shapes): B=64, N=32 query tokens, S=1024 doc tokens, D=128.
  pos/neg paired MaxSim + in-batch (b x c) MaxSim cross-entropy, T=0.02.

Strategy (v2 — 4-engine drain balance):
  * Shard the in-batch score matrix by DOC COLUMNS: core r computes
    scores[:, r*8:(r+1)*8] (all 64 query rows vs its 8 docs) plus the paired
    neg scores for its own 8 batch rows; pos_scores[b] == scores[b, b].
  * Every (doc, mtile) tile's 1024 dot products land in a 2-bank PSUM tile
    and must drain through ACT or DVE (the only PSUM readers). Three routes
    keep all four engines busy:
    - T (docs 0-3 + negs, host sum/diff prep): P=q@hsum, Q=q@hdif; ACT abs
      PSUM->SBUF, PE identity-matmul merges |Q| onto P, DVE max-reduces the
      512-wide merged bank. (baseline route)
    - S (raw halves): DVE scalar_tensor_tensor computes max(A,B) into SBUF
      in 512 cycles (2 PSUM streams at once), Pool tensor_tensor max-tree
      512->256->128, DVE 128-wide final reduce.
    - PT (raw halves): ACT copies both banks (one 1024-wide instr) to SBUF,
      Pool max-tree 1024->512->256, DVE 256-wide final reduce.
  * Per-(qtok, doc) maxes accumulate into SBUF mx[128, 136] (col = doc*16
    + mtile; neg cols at 128+). mx is DMA'd out; the n-sum over 32 query
    tokens and the O(64x64) softmax/softplus epilogue run on host.
"""

import numpy as np

B = 64
N = 32  # query tokens per row
S = 1024  # doc tokens
D = 128
NCORES = 8
LB = B // NCORES  # 8 docs (and batch rows) per core
H = S // 2  # 512, half-doc
MT = (B * N) // 128  # 16 m-tiles of 128 query tokens
TEMP = 0.02
NTILES = LB * MT  # 128 in-batch tiles per core
OUT_COLS = NTILES + LB  # 128 in-batch cols + 8 neg cols = 136
NPREP = 7  # docs 0..NPREP-1 take route T (host sum/diff prep)

_NC_CACHE = {}


def _build_nc():
    import concourse.bacc as bacc
    import concourse.mybir as mybir
    import concourse.tile as tile

    F32 = mybir.dt.float32
    F32R = mybir.dt.float32r
    X = mybir.AxisListType.X
    ABS = mybir.ActivationFunctionType.Abs
    MAX = mybir.AluOpType.max
    ADD = mybir.AluOpType.add

    nc = bacc.Bacc("TRN2", target_bir_lowering=False, debug=False)

    qT = nc.dram_tensor("qT", [128, B * N], F32, kind="ExternalInput").ap()
    qLocT = nc.dram_tensor("qLocT", [128, LB * N], F32, kind="ExternalInput").ap()
    dsumT = nc.dram_tensor("dsumT", [128, NPREP * H], F32, kind="ExternalInput").ap()
    ddifT = nc.dram_tensor("ddifT", [128, NPREP * H], F32, kind="ExternalInput").ap()
    rawT = nc.dram_tensor("rawT", [128, (LB - NPREP) * S], F32, kind="ExternalInput").ap()
    nsumT = nc.dram_tensor("nsumT", [128, LB * H], F32, kind="ExternalInput").ap()
    ndifT = nc.dram_tensor("ndifT", [128, LB * H], F32, kind="ExternalInput").ap()
    iden = nc.dram_tensor("iden", [128, 128], F32, kind="ExternalInput").ap()
    out = nc.dram_tensor("out", [128, OUT_COLS], F32, kind="ExternalOutput").ap()

    with tile.TileContext(nc) as tc:
        with (
            tc.tile_pool(name="consts", bufs=1) as consts,
            tc.tile_pool(name="docs", bufs=1) as docs_p,
            tc.tile_pool(name="absq", bufs=6) as absq_p,
            tc.tile_pool(name="psump", bufs=4, space="PSUM") as psum_pp,
        ):
            q_t = []
            for g in range(4):
                t = consts.tile([128, 512], F32R, tag=f"q{g}")
                q_t.append(t)
            id_t = consts.tile([128, 128], F32R, tag="id")
            ql_t = consts.tile([128, LB * N], F32R, tag="ql")
            mx = consts.tile([128, OUT_COLS], F32, tag="mx")
            nc.vector.memset(mx[:], 0.0)

            # HAM warm-up: dummy matmuls on memset data while input DMAs are
            # still in flight, so real matmuls start at the full PE clock
            wa = consts.tile([128, 128], F32, tag="wa")
            nc.vector.memset(wa[:], 0.0)
            wps = psum_pp.tile([128, 1024], F32, tag="pp", name="warm")
            for _ in range(12):
                nc.tensor.matmul(wps[:, 0:128], wa[:], wa[:], start=True, stop=True)

            # prepped docs 0-3 (sum/dif halves); raw docs 4-7; negs prepped
            ds0 = docs_p.tile([128, H], F32R, tag="ds0")
            dd0 = docs_p.tile([128, H], F32R, tag="dd0")
            dsR = docs_p.tile([128, (NPREP - 1) * H], F32R, tag="dsR")
            ddR = docs_p.tile([128, (NPREP - 1) * H], F32R, tag="ddR")
            rw0 = docs_p.tile([128, S], F32R, tag="rw0")
            nsr = docs_p.tile([128, LB * H], F32R, tag="nsr")
            ndr = docs_p.tile([128, LB * H], F32R, tag="ndr")

            def ds_ap(c):
                return ds0[:] if c == 0 else dsR[:, (c - 1) * H : c * H]

            def dd_ap(c):
                return dd0[:] if c == 0 else ddR[:, (c - 1) * H : c * H]

            def raw_ap(c):  # c == 7 only
                return rw0[:]

            nc.sync.dma_start(ds0[:], dsumT[:, 0:H].bitcast(F32R))
            nc.sync.dma_start(dd0[:], ddifT[:, 0:H].bitcast(F32R))
            nc.sync.dma_start(q_t[0][:], qT[:, 0:512].bitcast(F32R))
            nc.sync.dma_start(id_t[:], iden[:].bitcast(F32R))
            nc.sync.dma_start(rw0[:], rawT[:, 0:S].bitcast(F32R))
            for g in range(1, 4):
                nc.sync.dma_start(q_t[g][:], qT[:, g * 512 : (g + 1) * 512].bitcast(F32R))
            nc.sync.dma_start(dsR[:, 0 : 3 * H], dsumT[:, H : 4 * H].bitcast(F32R))
            nc.sync.dma_start(dsR[:, 3 * H : 6 * H], dsumT[:, 4 * H : 7 * H].bitcast(F32R))
            nc.sync.dma_start(ddR[:, 0 : 3 * H], ddifT[:, H : 4 * H].bitcast(F32R))
            nc.sync.dma_start(ddR[:, 3 * H : 6 * H], ddifT[:, 4 * H : 7 * H].bitcast(F32R))
            nc.sync.dma_start(ql_t[:], qLocT[:].bitcast(F32R))
            nc.sync.dma_start(nsr[:], nsumT[:].bitcast(F32R))
            nc.sync.dma_start(ndr[:], ndifT[:].bitcast(F32R))

            # route T software pipeline: defer each tile's identity-merge
            # matmul and reduce by one tile so PE never waits on ScalarE abs
            pend = []

            def flush_pend():
                pban, aq, idw, colw, parts = pend.pop(0)
                nc.tensor.matmul(
                    pban[0:parts, 0:H], idw, aq, start=False, stop=True
                )
                nc.vector.reduce_max(
                    mx[0:parts, colw : colw + 1],
                    pban[0:parts, 0:H].rearrange("p (w k) -> p w k", w=1),
                    axis=X,
                )

            def emit_T(lhs, sum_ap, dif_ap, col, parts):
                pban = psum_pp.tile([128, 1024], F32, tag="pp")
                nc.tensor.matmul(pban[0:parts, 0:H], lhs, sum_ap, start=True, stop=False)
                nc.tensor.matmul(pban[0:parts, H:S], lhs, dif_ap, start=True, stop=True)
                aq = absq_p.tile([128, H], F32R, tag="aq")
                nc.scalar.activation(aq[0:parts, :], pban[0:parts, H:S], ABS)
                if pend:
                    flush_pend()
                idw = id_t[0:parts, 0:parts]
                pend.append((pban, aq[0:parts, :], idw, col, parts))

            def emit_raw(lhs, raw_doc_ap, col):
                pban = psum_pp.tile([128, 1024], F32, tag="pp")
                nc.tensor.matmul(pban[:, 0:H], lhs, raw_doc_ap[:, 0:H], start=True, stop=True)
                nc.tensor.matmul(pban[:, H:S], lhs, raw_doc_ap[:, H:S], start=True, stop=True)
                nc.vector.reduce_max(
                    mx[:, col : col + 1],
                    pban[:].rearrange("p (w k) -> p w k", w=1),
                    axis=X,
                )

            # emit T tiles for prepped docs, injecting one raw (R) tile
            # after every 7th so the heavier R reduces spread across DVE
            work_T = [(c, m) for c in range(NPREP) for m in range(MT)]
            work_R = [(7, m) for m in range(MT)]
            seq = []
            for i, cm in enumerate(work_T):
                seq.append(("T", cm))
                if i % 7 == 6 and work_R:
                    seq.append(("R", work_R.pop(0)))
            for cm in work_R:
                seq.append(("R", cm))
            for kind, (c, m) in seq:
                col = c * MT + m
                lhs = q_t[m // 4][:, (m % 4) * 128 : (m % 4 + 1) * 128]
                if kind == "T":
                    emit_T(lhs, ds_ap(c), dd_ap(c), col, 128)
                else:
                    emit_raw(lhs, raw_ap(c), col)

            # paired neg term (route T, 32 partitions)
            for b in range(LB):
                lhs = ql_t[:, b * N : (b + 1) * N]
                emit_T(
                    lhs,
                    nsr[:, b * H : (b + 1) * H],
                    ndr[:, b * H : (b + 1) * H],
                    NTILES + b,
                    N,
                )
            while pend:
                flush_pend()

            nc.sync.dma_start(out[:], mx[:])

    nc.compile()
    return nc


def get_nc():
    if "nc" not in _NC_CACHE:
        _NC_CACHE["nc"] = _build_nc()
    return _NC_CACHE["nc"]


def _prep_inputs(q, d, nd):
    """Build the 8 per-core input maps."""
    qtok = np.ascontiguousarray(q.reshape(B * N, D).T)  # (128, 2048)
    iden = np.eye(128, dtype=np.float32)

    def halves(x):  # (B, S, D) -> sum/diff halves (B, 512, D)
        a = x[:, :H, :]
        b = x[:, H:, :]
        return (a + b) * np.float32(0.5), (a - b) * np.float32(0.5)

    hs, hd = halves(d)
    gs, gd = halves(nd)

    def chunkT(x, rows):  # (B, w, D) rows -> (128, len(rows)*w)
        c = x[rows]
        w = c.shape[1]
        return np.ascontiguousarray(np.transpose(c, (2, 0, 1)).reshape(D, len(rows) * w))

    maps = []
    for r in range(NCORES):
        rows = np.arange(r * LB, (r + 1) * LB)
        maps.append(
            {
                "qT": qtok,
                "qLocT": np.ascontiguousarray(qtok[:, r * LB * N : (r + 1) * LB * N]),
                "dsumT": chunkT(hs, rows[:NPREP]),
                "ddifT": chunkT(hd, rows[:NPREP]),
                "rawT": chunkT(d, rows[NPREP:]),
                "nsumT": chunkT(gs, rows),
                "ndifT": chunkT(gd, rows),
                "iden": iden,
            }
        )
    return maps


def _epilogue(blocks, offset):
    """blocks: list of 8 (128, OUT_COLS) arrays -> final loss (float32)."""
    S_mat = np.empty((B, B), dtype=np.float64)
    negs = np.empty(B, dtype=np.float64)
    for r in range(NCORES):
        blk = np.asarray(blocks[r], dtype=np.float64)
        # blk[p, c*MT + m] = max for query token (m*128 + p) vs doc (r*LB + c)
        ib = blk[:, :NTILES].reshape(128, LB, MT)  # (p, c, m)
        tok = np.transpose(ib, (2, 0, 1)).reshape(B * N, LB)  # (global tok, c)
        S_mat[:, r * LB : (r + 1) * LB] = tok.reshape(B, N, LB).sum(axis=1)
        # blk[p, NTILES + b] = max for token p (p<32) of row r*LB+b vs its neg
        negs[r * LB : (r + 1) * LB] = blk[:N, NTILES:].sum(axis=0)

    pos = np.diag(S_mat)
    x = (negs - pos) / TEMP
    loss1 = np.logaddexp(0.0, x).mean()  # stable softplus

    logits = S_mat / TEMP
    # jnp.take_along_axis index semantics: negative indices wrap once,
    # out-of-range indices yield NaN (fill mode)
    raw = np.arange(B) + int(offset)
    idx = np.where(raw < 0, raw + B, raw)
    valid = (idx >= 0) & (idx < B)
    row_max = logits.max(axis=1, keepdims=True)
    lse = np.log(np.exp(logits - row_max).sum(axis=1, keepdims=True)) + row_max
    logp = logits - lse
    picked = logp[np.arange(B), np.clip(idx, 0, B - 1)]
    picked = np.where(valid, picked, np.nan)
    ce = -picked.mean()

    return np.float32((loss1 + ce) / 2.0)


def kernel(query_embeddings, doc_embeddings, neg_doc_embeddings, offset):
    from concourse.bass_utils import run_bass_kernel_spmd

    q = np.asarray(query_embeddings, dtype=np.float32)
    d = np.asarray(doc_embeddings, dtype=np.float32)
    nd = np.asarray(neg_doc_embeddings, dtype=np.float32)
    assert q.shape == (B, N, D) and d.shape == (B, S, D) and nd.shape == (B, S, D)

    nc = get_nc()
    maps = _prep_inputs(q, d, nd)
    res = run_bass_kernel_spmd(nc, maps, core_ids=list(range(NCORES)))
    blocks = [res.results[r]["out"] for r in range(NCORES)]
    return _epilogue(blocks, offset)


def run_traced(query_embeddings, doc_embeddings, neg_doc_embeddings, offset, **trace_kw):
    """Like kernel() but returns (loss, BassKernelResults) for profiling."""
    from concourse.bass_utils import run_bass_kernel_spmd

    q = np.asarray(query_embeddings, dtype=np.float32)
    d = np.asarray(doc_embeddings, dtype=np.float32)
    nd = np.asarray(neg_doc_embeddings, dtype=np.float32)
    nc = get_nc()
    maps = _prep_inputs(q, d, nd)
    res = run_bass_kernel_spmd(
        nc, maps, core_ids=list(range(NCORES)), trace=True, **trace_kw
    )
    blocks = [res.results[r]["out"] for r in range(NCORES)]
    return _epilogue(blocks, offset), res
